# revision 1
# baseline (speedup 1.0000x reference)
"""Trainium2 kernel for nn_ImageStitchingLayer: 2x2 stitching NCC loss.

Math: for z_weights in [0,1), the reference's z-interpolation is a 2-tap blend
s[k] = (1-f)*x[k] + f*x[k-1] (zero-padded to Z+1 planes).  Every sum in the
NCC loss then decomposes into z-lag Gram statistics of the raw overlap slabs:

    sum(s)        = S                    (independent of f)
    sum(s^2)      = ((1-f)^2 + f^2) A + 2 f (1-f) B
    sum(s1 s2)    = ((1-f1)(1-f2) + f1 f2) C0 + (1-f1) f2 Cp + f1 (1-f2) Cm

with S = sum(x), A = sum(x^2), B = sum(x[z] x[z-1]), C0/Cp/Cm the lag-0/+-1
cross sums.  All of these are entries of the z-by-z Gram matrix of the two
slabs, contracted over the 16k hw positions per core.  The device computes
the Gram matrices on the tensor engine (bf16 inputs, fp32 PSUM accumulation);
the host combines them in float64.

Sharding: 4 adjacent pairs x 2 half-slabs = 8 cores.  Each core receives only
its two overlap slab halves (48 x 64 x 256 x 2 each), packed host-side as
[128 partitions(hw) x (chunk, ch, [x1 z | x2 z | 1])] bf16.
"""

import numpy as np
import ml_dtypes

Z, H, W = 48, 512, 512
OH = 64
NCH = 2
PAIRS = [(1, 0, "h"), (2, 0, "v"), (3, 1, "v"), (3, 2, "h")]
NCORES = 8

USE_FP8 = False  # fp8-e4m3 halves DMA (~25us vs ~32us) but rel err 1.5e-3 vs 4e-6; keep bf16
ZCOLS = 2 * Z + 1  # 97: x1 z-planes | x2 z-planes | ones column
NCHUNK = 128  # hw chunks of 128 partitions each (16384 hw positions / core)
PIECES = 8  # DMA pieces (pipelining granularity)
CPP = NCHUNK // PIECES  # chunks per piece
PIECE_COLS = CPP * NCH * ZCOLS  # 3104
PIECE_PAD = 32  # so a 128-wide (FWL-eligible) lhsT never overruns
PIECE_W = PIECE_COLS + PIECE_PAD  # 3136
OUT_COLS = NCH * ZCOLS  # 194

_CACHE = {}

LAST_RESULT = None  # BassKernelResults of the most recent device run (for test harness)


def _build_bass():
    """Raw bass (no TileContext): this container's walrus rejects >3 sem waits on
    one instruction, which Tile's kernel-tail drain always exceeds.  Manual sync
    keeps every instruction at <=1 wait."""
    import concourse.bass as bass
    from concourse import mybir

    nc = bass.Bass()
    in_dt = mybir.dt.float8e4 if USE_FP8 else mybir.dt.bfloat16
    x = nc.dram_tensor("x", [PIECES, 128, PIECE_W], in_dt, kind="ExternalInput")
    out = nc.dram_tensor("out", [128, OUT_COLS], mybir.dt.float32, kind="ExternalOutput")

    with (
        nc.sbuf_tensor([128, PIECES * PIECE_W], in_dt) as data,
        nc.sbuf_tensor([128, OUT_COLS], mybir.dt.float32) as out_t,
        nc.psum_tensor([128, ZCOLS], mybir.dt.float32) as ps0,
        nc.psum_tensor([128, ZCOLS], mybir.dt.float32) as ps1,
        nc.semaphore() as dma_sem,
        nc.semaphore() as pe_sem,
        nc.semaphore() as dve_sem,
        nc.Block() as block,
    ):
        psums = [ps0, ps1]

        @block.sync
        def _(sync):
            for j in range(PIECES):
                sync.dma_start(
                    data[:, j * PIECE_W : (j + 1) * PIECE_W], x[j]
                ).then_inc(dma_sem, 16)
            sync.wait_ge(dve_sem, 1)
            sync.dma_start(out[:], out_t[:]).then_inc(dma_sem, 16)
            sync.wait_ge(dma_sem, (PIECES + 1) * 16)

        @block.tensor
        def _(tensor):
            for j in range(PIECES):
                tensor.wait_ge(dma_sem, (j + 1) * 16)
                for k in range(CPP):
                    g = j * CPP + k
                    for c in range(NCH):
                        base = j * PIECE_W + (k * NCH + c) * ZCOLS
                        mm = tensor.matmul(
                            psums[c][:, :],
                            data[:, base : base + 128],  # stationary (cols 97.. junk)
                            data[:, base : base + ZCOLS],  # moving [128hw, 97]
                            start=(g == 0),
                            stop=(g == NCHUNK - 1),
                        )
            mm.then_inc(pe_sem, 1)

        @block.vector
        def _(vector):
            vector.wait_ge(pe_sem, 1)
            vector.tensor_copy(out_t[:, 0:ZCOLS], ps0[:, :])
            vector.tensor_copy(out_t[:, ZCOLS : 2 * ZCOLS], ps1[:, :]).then_inc(
                dve_sem, 1
            )

    return nc


def _pack_core(x1, x2):
    """x1, x2: [Z, OH, 256, NCH] float32 -> [PIECES, 128, PIECE_W] bf16."""

    def r(x):  # -> [chunk, p, ch, z]
        # [Z, 64, 256, c] -> [64, 256, c, Z] -> [hw, c, Z] -> [chunk, p, c, Z]
        return np.ascontiguousarray(x.transpose(1, 2, 3, 0)).reshape(NCHUNK, 128, NCH, Z)

    dt = ml_dtypes.float8_e4m3 if USE_FP8 else ml_dtypes.bfloat16
    x1r = r(x1)
    x2r = r(x2)
    D = np.empty((128, NCHUNK, NCH, ZCOLS), dtype=dt)  # [p, k, c, q]
    D[:, :, :, 0:Z] = x1r.transpose(1, 0, 2, 3)
    D[:, :, :, Z : 2 * Z] = x2r.transpose(1, 0, 2, 3)
    D[:, :, :, 2 * Z] = 1.0
    X = np.zeros((PIECES, 128, PIECE_W), dtype=dt)
    X[:, :, :PIECE_COLS] = D.reshape(128, PIECES, PIECE_COLS).transpose(1, 0, 2)
    return X


def _slabs(stacks):
    """Yield (x1_half, x2_half) float32 views/copies per core, canonical [Z,64,512,2] split in two."""
    out = []
    for i, j, ori in PAIRS:
        if ori == "v":
            a = stacks[i][:, 0:OH, :, :]
            b = stacks[j][:, H - OH : H, :, :]
        else:
            a = stacks[i][:, :, 0:OH, :].transpose(0, 2, 1, 3)
            b = stacks[j][:, :, W - OH : W, :].transpose(0, 2, 1, 3)
        for half in range(2):
            sl = slice(half * 256, (half + 1) * 256)
            out.append((a[:, :, sl, :], b[:, :, sl, :]))
    return out


def _run_device(in_maps, trace=False):
    global LAST_RESULT
    from concourse import bass_utils

    if "nc" not in _CACHE:
        _CACHE["nc"] = _build_bass()
    for _attempt in range(3):
        res = bass_utils.run_bass_kernel_spmd(
            _CACHE["nc"], in_maps, core_ids=list(range(NCORES)), trace=trace
        )
        LAST_RESULT = res
        ok = all(np.isfinite(r["out"]).all() and np.abs(r["out"]).sum() > 0 for r in res.results)
        if ok:
            break
    return res.results


def kernel(stacks, z_weights):
    stacks = np.asarray(stacks, dtype=np.float32)
    zw = np.asarray(z_weights, dtype=np.float64)

    in_maps = [{"x": _pack_core(x1, x2)} for (x1, x2) in _slabs(stacks)]
    results = _run_device(in_maps)

    N = (Z + 1) * OH * W
    loss = 0.0
    for p_idx, (i, j, _ori) in enumerate(PAIRS):
        f1, f2 = zw[i], zw[j]
        O = results[2 * p_idx]["out"].astype(np.float64) + results[2 * p_idx + 1][
            "out"
        ].astype(np.float64)
        for c in range(NCH):
            M = O[:, c * ZCOLS : (c + 1) * ZCOLS]
            G11 = M[0:Z, 0:Z]
            G12 = M[0:Z, Z : 2 * Z]
            G22 = M[Z : 2 * Z, Z : 2 * Z]
            S1 = M[0:Z, 2 * Z].sum()
            S2 = M[Z : 2 * Z, 2 * Z].sum()
            A1 = np.trace(G11)
            B1 = np.trace(G11, offset=-1)
            A2 = np.trace(G22)
            B2 = np.trace(G22, offset=-1)
            C0 = np.trace(G12)
            Cp = np.trace(G12, offset=-1)  # sum_z x1[z] x2[z-1]
            Cm = np.trace(G12, offset=1)  # sum_z x1[z-1] x2[z]
            ss1 = ((1 - f1) ** 2 + f1**2) * A1 + 2 * f1 * (1 - f1) * B1
            ss2 = ((1 - f2) ** 2 + f2**2) * A2 + 2 * f2 * (1 - f2) * B2
            s12 = (
                ((1 - f1) * (1 - f2) + f1 * f2) * C0
                + (1 - f1) * f2 * Cp
                + f1 * (1 - f2) * Cm
            )
            m11 = ss1 - S1 * S1 / N
            m22 = ss2 - S2 * S2 / N
            m12 = s12 - S1 * S2 / N
            loss += m12**2 + m11 * m22

    return np.array(loss, dtype=np.float32)



# revision 3
# speedup vs baseline: 1.2893x; 1.2893x over previous
"""Trainium2 kernel for nn_ImageStitchingLayer: 2x2 stitching NCC loss.

Math: for z_weights in [0,1), the reference's z-interpolation is a 2-tap blend
s[k] = (1-f)*x[k] + f*x[k-1] (zero-padded to Z+1 planes).  Every sum in the
NCC loss then decomposes into z-lag Gram statistics of the raw overlap slabs:

    sum(s)        = S                    (independent of f)
    sum(s^2)      = ((1-f)^2 + f^2) A + 2 f (1-f) B
    sum(s1 s2)    = ((1-f1)(1-f2) + f1 f2) C0 + (1-f1) f2 Cp + f1 (1-f2) Cm

with S = sum(x), A = sum(x^2), B = sum(x[z] x[z-1]), C0/Cp/Cm the lag-0/+-1
cross sums.  All are entries of the z-by-z Gram matrix of the two slabs,
contracted over hw positions.  The device computes the Gram matrices on the
tensor engine (fp8-e4m3 inputs, fp32 PSUM accumulation); the host combines
them in float64.  fp8 end-to-end loss error ~1.5e-3 (gate is 2e-2); DMA is
half of bf16 and the kernel is then tensor-engine-bound.

Sharding: 4 adjacent pairs x 2 half-slabs = 8 cores.  Each core receives its
two overlap slab halves (48 x 64 x 256 x 2 each) packed host-side as
[128 partitions(hw) x chunks x ch x (ones | x1 z's | x2 z's)] fp8.

Perf notes (vs the bf16 v1 at ~34.5us):
 - fp8 halves the HBM stream (6.4MB -> 3.2MB/core); the single SP HWDGE
   queue already runs at the ~400GB/s roofline, so the PE (1 moving col per
   cycle Gram) becomes the bottleneck at ~10-12us.
 - moving operand drops the ones column (96 cols, sums come from the
   stationary ones row), stationary stays 128-wide for fast weight load.
 - a few warm-up matmuls on garbage data ramp the PE out of its low p-state
   during the ~7us fixed NEFF/DMA-issue preamble.
 - first DMA piece is small (4 chunks) so real matmuls start earlier.
 - device Gram row-0/diagonal sums are checked against host-side sums;
   mismatch (rare transient device glitch) triggers a re-run.
"""

import numpy as np
import ml_dtypes

Z, H, W = 48, 512, 512
OH = 64
NCH = 2
PAIRS = [(1, 0, "h"), (2, 0, "v"), (3, 1, "v"), (3, 2, "h")]
NCORES = 8

CB = Z * 2 + 1  # 97: per-channel block = [ones | x1 z0..47 | x2 z0..47]
NMOV = 2 * Z  # 96 moving columns (data only; ones row comes from stationary)
CHUNK_COLS = NCH * CB  # 194 cols per 128-hw chunk
NCHUNK = 128  # 16384 hw positions / core
PIECE_CHUNKS = [4, 12, 16, 16, 16, 16, 16, 16, 16]  # 128 total; small first piece
NPIECES = len(PIECE_CHUNKS)
TAIL_PAD = 32  # so the last chunk's 128-wide stationary never overruns
TOT_COLS = NCHUNK * CHUNK_COLS + TAIL_PAD
OUT_COLS = NCH * NMOV  # 192
N_WARM_MM = 4  # p-state warm-up matmuls (N=512 each, ~0.4-0.6us apiece cold)

_CACHE = {}

LAST_RESULT = None  # BassKernelResults of the most recent device run (for test harness)


def _build_bass():
    """Raw bass (no TileContext): this container's walrus rejects >3 sem waits on
    one instruction, which Tile's kernel-tail drain always exceeds.  Manual sync
    keeps every instruction at <=1 wait."""
    import concourse.bass as bass
    from concourse import mybir

    nc = bass.Bass()
    in_dt = mybir.dt.float8e4

    piece_cols = [n * CHUNK_COLS for n in PIECE_CHUNKS]
    piece_off = np.cumsum([0] + piece_cols).tolist()
    xs = [
        nc.dram_tensor(f"x{j}", [128, piece_cols[j]], in_dt, kind="ExternalInput")
        for j in range(NPIECES)
    ]
    out = nc.dram_tensor("out", [128, OUT_COLS], mybir.dt.float32, kind="ExternalOutput")

    with (
        nc.sbuf_tensor([128, TOT_COLS], in_dt) as data,
        nc.sbuf_tensor([128, OUT_COLS], mybir.dt.float32) as out_t,
        nc.psum_tensor([128, NMOV], mybir.dt.float32) as ps0,
        nc.psum_tensor([128, NMOV], mybir.dt.float32) as ps1,
        nc.psum_tensor([128, 512], mybir.dt.float32) as ps_warm,
        nc.semaphore() as dma_sem,
        nc.semaphore() as pe_sem,
        nc.semaphore() as dve_sem,
        nc.Block(no_gpsimd_drain=True) as block,
    ):
        psums = [ps0, ps1]

        @block.sync
        def _(sync):
            for j in range(NPIECES):
                sync.dma_start(
                    data[:, piece_off[j] : piece_off[j + 1]], xs[j][:, :]
                ).then_inc(dma_sem, 16)
            sync.wait_ge(dve_sem, 1)
            sync.dma_start(out[:], out_t[:]).then_inc(dma_sem, 16)
            sync.wait_ge(dma_sem, (NPIECES + 1) * 16)

        @block.tensor
        def _(tensor):
            # p-state warm-up on whatever garbage is in the (not yet DMA'd)
            # tail of the data tile; results go to a scratch PSUM bank.
            warm0 = piece_off[-2]
            for _ in range(N_WARM_MM):
                tensor.matmul(
                    ps_warm[:, :],
                    data[:, warm0 : warm0 + 128],
                    data[:, warm0 : warm0 + 512],
                    start=True,
                    stop=True,
                )
            g = 0
            for j in range(NPIECES):
                tensor.wait_ge(dma_sem, (j + 1) * 16)
                for k in range(PIECE_CHUNKS[j]):
                    for c in range(NCH):
                        base = piece_off[j] + (k * NCH + c) * CB
                        mm = tensor.matmul(
                            psums[c][:, :],
                            data[:, base : base + 128],  # [ones|x1|x2|junk] 128-wide (FWL)
                            data[:, base + 1 : base + 1 + NMOV],  # [x1|x2] 96 cols
                            start=(g == 0),
                            stop=(g == NCHUNK - 1),
                        )
                    g += 1
            mm.then_inc(pe_sem, 1)

        @block.vector
        def _(vector):
            vector.wait_ge(pe_sem, 1)
            vector.tensor_copy(out_t[:, 0:NMOV], ps0[:, :])
            vector.tensor_copy(out_t[:, NMOV : 2 * NMOV], ps1[:, :]).then_inc(
                dve_sem, 1
            )

    return nc


def _pack_core(x1, x2):
    """x1, x2: [Z, OH, 256, NCH] float32 -> dict of per-piece [128, cols] fp8."""

    def r(x):  # -> [chunk, part, ch, z]
        return np.ascontiguousarray(x.transpose(1, 2, 3, 0)).reshape(NCHUNK, 128, NCH, Z)

    dt = ml_dtypes.float8_e4m3
    x1r = r(x1)
    x2r = r(x2)
    D = np.empty((128, NCHUNK, NCH, CB), dtype=dt)  # [part, chunk, ch, cb]
    D[:, :, :, 0] = 1.0
    D[:, :, :, 1 : 1 + Z] = x1r.transpose(1, 0, 2, 3)
    D[:, :, :, 1 + Z : CB] = x2r.transpose(1, 0, 2, 3)
    flat = D.reshape(128, NCHUNK * CHUNK_COLS)
    off = 0
    m = {}
    for j, n in enumerate(PIECE_CHUNKS):
        cols = n * CHUNK_COLS
        m[f"x{j}"] = np.ascontiguousarray(flat[:, off : off + cols])
        off += cols
    return m


def _slabs(stacks):
    """(x1_half, x2_half) float32 per core: canonical [Z,64,512,2] split in two."""
    out = []
    for i, j, ori in PAIRS:
        if ori == "v":
            a = stacks[i][:, 0:OH, :, :]
            b = stacks[j][:, H - OH : H, :, :]
        else:
            a = stacks[i][:, :, 0:OH, :].transpose(0, 2, 1, 3)
            b = stacks[j][:, :, W - OH : W, :].transpose(0, 2, 1, 3)
        for half in range(2):
            sl = slice(half * 256, (half + 1) * 256)
            out.append((a[:, :, sl, :], b[:, :, sl, :]))
    return out


def _expected_sums(in_map):
    """Per-channel expected [S_z row (96), diag (96)] from the packed fp8 data,
    in f64 — used to validate the device Gram against transient glitches."""
    full = np.concatenate(
        [in_map[f"x{j}"].astype(np.float64) for j in range(NPIECES)], axis=1
    )
    cols = full.reshape(128, NCHUNK, NCH, CB)
    colsum = cols.sum(axis=(0, 1))  # [NCH, CB]
    sqsum = (cols * cols).sum(axis=(0, 1))  # [NCH, CB]
    return colsum[:, 1:CB], sqsum[:, 1:CB]  # each [NCH, 96]


def _check_core(out_arr, colsum, sqsum):
    O = out_arr.astype(np.float64)
    for c in range(NCH):
        M = O[:, c * NMOV : (c + 1) * NMOV]
        dev_s = M[0, :]
        dev_d = M[1:97, :].diagonal()
        if not (
            np.allclose(dev_s, colsum[c], rtol=0, atol=2.0)
            and np.allclose(dev_d, sqsum[c], rtol=5e-3, atol=2.0)
        ):
            return False
    return True


def _run_device(in_maps, trace=False):
    global LAST_RESULT
    from concourse import bass_utils

    if "nc" not in _CACHE:
        _CACHE["nc"] = _build_bass()
    checks = [_expected_sums(m) for m in in_maps]
    for _attempt in range(4):
        res = bass_utils.run_bass_kernel_spmd(
            _CACHE["nc"], in_maps, core_ids=list(range(NCORES)), trace=trace
        )
        LAST_RESULT = res
        ok = all(
            np.isfinite(r["out"]).all() and _check_core(r["out"], cs, sq)
            for r, (cs, sq) in zip(res.results, checks)
        )
        if ok:
            break
    return res.results


def kernel(stacks, z_weights):
    stacks = np.asarray(stacks, dtype=np.float32)
    zw = np.asarray(z_weights, dtype=np.float64)

    in_maps = [_pack_core(x1, x2) for (x1, x2) in _slabs(stacks)]
    results = _run_device(in_maps)

    N = (Z + 1) * OH * W
    loss = 0.0
    for p_idx, (i, j, _ori) in enumerate(PAIRS):
        f1, f2 = zw[i], zw[j]
        O = results[2 * p_idx]["out"].astype(np.float64) + results[2 * p_idx + 1][
            "out"
        ].astype(np.float64)
        for c in range(NCH):
            M = O[:, c * NMOV : (c + 1) * NMOV]
            # rows: 0 = ones, 1..48 = x1 z's, 49..96 = x2 z's; cols: x1 z's | x2 z's
            G11 = M[1 : 1 + Z, 0:Z]
            G12 = M[1 : 1 + Z, Z : 2 * Z]
            G22 = M[1 + Z : 1 + 2 * Z, Z : 2 * Z]
            S1 = M[0, 0:Z].sum()
            S2 = M[0, Z : 2 * Z].sum()
            A1 = np.trace(G11)
            B1 = np.trace(G11, offset=-1)
            A2 = np.trace(G22)
            B2 = np.trace(G22, offset=-1)
            C0 = np.trace(G12)
            Cp = np.trace(G12, offset=-1)  # sum_z x1[z] x2[z-1]
            Cm = np.trace(G12, offset=1)  # sum_z x1[z-1] x2[z]
            ss1 = ((1 - f1) ** 2 + f1**2) * A1 + 2 * f1 * (1 - f1) * B1
            ss2 = ((1 - f2) ** 2 + f2**2) * A2 + 2 * f2 * (1 - f2) * B2
            s12 = (
                ((1 - f1) * (1 - f2) + f1 * f2) * C0
                + (1 - f1) * f2 * Cp
                + f1 * (1 - f2) * Cm
            )
            m11 = ss1 - S1 * S1 / N
            m22 = ss2 - S2 * S2 / N
            m12 = s12 - S1 * S2 / N
            loss += m12**2 + m11 * m22

    return np.array(loss, dtype=np.float32)


# revision 5
# speedup vs baseline: 1.3673x; 1.0605x over previous
"""Trainium2 kernel for nn_ImageStitchingLayer: 2x2 stitching NCC loss.

Math: for z_weights in [0,1), the reference's z-interpolation is a 2-tap blend
s[k] = (1-f)*x[k] + f*x[k-1] (zero-padded to Z+1 planes).  Every sum in the
NCC loss then decomposes into z-lag Gram statistics of the raw overlap slabs:

    sum(s)        = S                    (independent of f)
    sum(s^2)      = ((1-f)^2 + f^2) A + 2 f (1-f) B
    sum(s1 s2)    = ((1-f1)(1-f2) + f1 f2) C0 + (1-f1) f2 Cp + f1 (1-f2) Cm

with S = sum(x), A = sum(x^2), B = sum(x[z] x[z-1]), C0/Cp/Cm the lag-0/+-1
cross sums.  All are entries of the z-by-z Gram matrix of the two slabs,
contracted over hw positions.  The device computes the Gram matrices on the
tensor engine (fp8-e4m3 inputs, fp32 PSUM accumulation); the host combines
them in float64.  fp8 end-to-end loss error ~1.5e-3 (gate is 2e-2); DMA is
half of bf16 and the kernel is then tensor-engine-bound.

Sharding: 4 adjacent pairs x 2 half-slabs = 8 cores.  Each core receives its
two overlap slab halves (48 x 64 x 256 x 2 each) packed host-side as
[128 partitions(hw) x chunks x ch x (ones | x1 z's | x2 z's)] fp8.

Perf notes (vs the bf16 v1 at ~34.5us):
 - fp8 halves the HBM stream (6.4MB -> 3.2MB/core); the single SP HWDGE
   queue already runs at the ~400GB/s roofline, so the PE (1 moving col per
   cycle Gram) becomes the bottleneck at ~10-12us.
 - moving operand drops the ones column (96 cols, sums come from the
   stationary ones row), stationary stays 128-wide for fast weight load.
 - a few warm-up matmuls on garbage data ramp the PE out of its low p-state
   during the ~7us fixed NEFF/DMA-issue preamble.
 - first DMA piece is small (4 chunks) so real matmuls start earlier.
 - device Gram row-0/diagonal sums are checked against host-side sums;
   mismatch (rare transient device glitch) triggers a re-run.
"""

import numpy as np
import ml_dtypes

Z, H, W = 48, 512, 512
OH = 64
NCH = 2
PAIRS = [(1, 0, "h"), (2, 0, "v"), (3, 1, "v"), (3, 2, "h")]
NCORES = 8

CB = Z * 2 + 1  # 97: per-channel block = [ones | x1 z0..47 | x2 z0..47]
NMOV = 2 * Z  # 96 moving columns (data only; ones row comes from stationary)
CHUNK_COLS = NCH * CB  # 194 cols per 128-hw chunk
NCHUNK = 128  # 16384 hw positions / core
PIECE_CHUNKS = [4, 12, 16, 16, 16, 16, 16, 16, 16]  # 128 total; small first piece
NPIECES = len(PIECE_CHUNKS)
TAIL_PAD = 32  # so the last chunk's 128-wide stationary never overruns
TOT_COLS = NCHUNK * CHUNK_COLS + TAIL_PAD
OUT_COLS = NCH * NMOV  # 192
N_WARM_MM = 6  # p-state warm-up matmuls (N=512 each); sized so the last one
# ends right as DMA piece 0's completion semaphore arrives — a gap there
# resets the PE DVFS ramp and the first ~70 real matmuls run at 1.2 GHz.

_CACHE = {}

LAST_RESULT = None  # BassKernelResults of the most recent device run (for test harness)


def _build_bass():
    """Raw bass (no TileContext): this container's walrus rejects >3 sem waits on
    one instruction, which Tile's kernel-tail drain always exceeds.  Manual sync
    keeps every instruction at <=1 wait."""
    import concourse.bass as bass
    from concourse import mybir

    nc = bass.Bass()
    # The kernel only issues DMAs from the SP (sync) engine; dropping the
    # unused Activation HWDGE queue set (16 rings) trims the NRT queue
    # init/teardown that is counted inside the NEFF execution window.
    nc.m.queues = [q for q in nc.m.queues if q.name != "qScalarDynamicHW"]
    in_dt = mybir.dt.float8e4

    piece_cols = [n * CHUNK_COLS for n in PIECE_CHUNKS]
    piece_off = np.cumsum([0] + piece_cols).tolist()
    xs = [
        nc.dram_tensor(f"x{j}", [128, piece_cols[j]], in_dt, kind="ExternalInput")
        for j in range(NPIECES)
    ]
    out = nc.dram_tensor("out", [128, OUT_COLS], mybir.dt.float32, kind="ExternalOutput")

    with (
        nc.sbuf_tensor([128, TOT_COLS], in_dt) as data,
        nc.sbuf_tensor([128, OUT_COLS], mybir.dt.float32) as out_t,
        nc.psum_tensor([128, NMOV], mybir.dt.float32) as ps0,
        nc.psum_tensor([128, NMOV], mybir.dt.float32) as ps1,
        nc.psum_tensor([128, 512], mybir.dt.float32) as ps_warm,
        nc.semaphore() as dma_sem,
        nc.semaphore() as pe_sem,
        nc.semaphore() as dve_sem,
        nc.Block(no_gpsimd_drain=True) as block,
    ):
        psums = [ps0, ps1]

        @block.sync
        def _(sync):
            for j in range(NPIECES):
                sync.dma_start(
                    data[:, piece_off[j] : piece_off[j + 1]], xs[j][:, :]
                ).then_inc(dma_sem, 16)
            sync.wait_ge(dve_sem, 1)
            sync.dma_start(out[:], out_t[:]).then_inc(dma_sem, 16)
            sync.wait_ge(dma_sem, (NPIECES + 1) * 16)

        @block.tensor
        def _(tensor):
            # p-state warm-up on whatever garbage is in the (not yet DMA'd)
            # tail of the data tile; results go to a scratch PSUM bank.
            warm0 = piece_off[-2]
            for _ in range(N_WARM_MM):
                tensor.matmul(
                    ps_warm[:, :],
                    data[:, warm0 : warm0 + 128],
                    data[:, warm0 : warm0 + 512],
                    start=True,
                    stop=True,
                )
            g = 0
            for j in range(NPIECES):
                tensor.wait_ge(dma_sem, (j + 1) * 16)
                for k in range(PIECE_CHUNKS[j]):
                    for c in range(NCH):
                        base = piece_off[j] + (k * NCH + c) * CB
                        mm = tensor.matmul(
                            psums[c][:, :],
                            data[:, base : base + 128],  # [ones|x1|x2|junk] 128-wide (FWL)
                            data[:, base + 1 : base + 1 + NMOV],  # [x1|x2] 96 cols
                            start=(g == 0),
                            stop=(g == NCHUNK - 1),
                        )
                    g += 1
            mm.then_inc(pe_sem, 1)

        @block.vector
        def _(vector):
            vector.wait_ge(pe_sem, 1)
            vector.tensor_copy(out_t[:, 0:NMOV], ps0[:, :])
            vector.tensor_copy(out_t[:, NMOV : 2 * NMOV], ps1[:, :]).then_inc(
                dve_sem, 1
            )

    return nc


def _pack_core(x1, x2):
    """x1, x2: [Z, OH, 256, NCH] float32 -> dict of per-piece [128, cols] fp8."""

    def r(x):  # -> [chunk, part, ch, z]
        return np.ascontiguousarray(x.transpose(1, 2, 3, 0)).reshape(NCHUNK, 128, NCH, Z)

    dt = ml_dtypes.float8_e4m3
    x1r = r(x1)
    x2r = r(x2)
    D = np.empty((128, NCHUNK, NCH, CB), dtype=dt)  # [part, chunk, ch, cb]
    D[:, :, :, 0] = 1.0
    D[:, :, :, 1 : 1 + Z] = x1r.transpose(1, 0, 2, 3)
    D[:, :, :, 1 + Z : CB] = x2r.transpose(1, 0, 2, 3)
    flat = D.reshape(128, NCHUNK * CHUNK_COLS)
    off = 0
    m = {}
    for j, n in enumerate(PIECE_CHUNKS):
        cols = n * CHUNK_COLS
        m[f"x{j}"] = np.ascontiguousarray(flat[:, off : off + cols])
        off += cols
    return m


def _slabs(stacks):
    """(x1_half, x2_half) float32 per core: canonical [Z,64,512,2] split in two."""
    out = []
    for i, j, ori in PAIRS:
        if ori == "v":
            a = stacks[i][:, 0:OH, :, :]
            b = stacks[j][:, H - OH : H, :, :]
        else:
            a = stacks[i][:, :, 0:OH, :].transpose(0, 2, 1, 3)
            b = stacks[j][:, :, W - OH : W, :].transpose(0, 2, 1, 3)
        for half in range(2):
            sl = slice(half * 256, (half + 1) * 256)
            out.append((a[:, :, sl, :], b[:, :, sl, :]))
    return out


def _expected_sums(in_map):
    """Per-channel expected [S_z row (96), diag (96)] from the packed fp8 data,
    in f64 — used to validate the device Gram against transient glitches."""
    full = np.concatenate(
        [in_map[f"x{j}"].astype(np.float64) for j in range(NPIECES)], axis=1
    )
    cols = full.reshape(128, NCHUNK, NCH, CB)
    colsum = cols.sum(axis=(0, 1))  # [NCH, CB]
    sqsum = (cols * cols).sum(axis=(0, 1))  # [NCH, CB]
    return colsum[:, 1:CB], sqsum[:, 1:CB]  # each [NCH, 96]


def _check_core(out_arr, colsum, sqsum):
    O = out_arr.astype(np.float64)
    for c in range(NCH):
        M = O[:, c * NMOV : (c + 1) * NMOV]
        dev_s = M[0, :]
        dev_d = M[1:97, :].diagonal()
        if not (
            np.allclose(dev_s, colsum[c], rtol=0, atol=2.0)
            and np.allclose(dev_d, sqsum[c], rtol=5e-3, atol=2.0)
        ):
            return False
    return True


def _run_device(in_maps, trace=False):
    global LAST_RESULT
    from concourse import bass_utils

    if "nc" not in _CACHE:
        _CACHE["nc"] = _build_bass()
    checks = [_expected_sums(m) for m in in_maps]
    for _attempt in range(4):
        res = bass_utils.run_bass_kernel_spmd(
            _CACHE["nc"], in_maps, core_ids=list(range(NCORES)), trace=trace
        )
        LAST_RESULT = res
        ok = all(
            np.isfinite(r["out"]).all() and _check_core(r["out"], cs, sq)
            for r, (cs, sq) in zip(res.results, checks)
        )
        if ok:
            break
    return res.results


def kernel(stacks, z_weights):
    stacks = np.asarray(stacks, dtype=np.float32)
    zw = np.asarray(z_weights, dtype=np.float64)

    in_maps = [_pack_core(x1, x2) for (x1, x2) in _slabs(stacks)]
    results = _run_device(in_maps)

    N = (Z + 1) * OH * W
    loss = 0.0
    for p_idx, (i, j, _ori) in enumerate(PAIRS):
        f1, f2 = zw[i], zw[j]
        O = results[2 * p_idx]["out"].astype(np.float64) + results[2 * p_idx + 1][
            "out"
        ].astype(np.float64)
        for c in range(NCH):
            M = O[:, c * NMOV : (c + 1) * NMOV]
            # rows: 0 = ones, 1..48 = x1 z's, 49..96 = x2 z's; cols: x1 z's | x2 z's
            G11 = M[1 : 1 + Z, 0:Z]
            G12 = M[1 : 1 + Z, Z : 2 * Z]
            G22 = M[1 + Z : 1 + 2 * Z, Z : 2 * Z]
            S1 = M[0, 0:Z].sum()
            S2 = M[0, Z : 2 * Z].sum()
            A1 = np.trace(G11)
            B1 = np.trace(G11, offset=-1)
            A2 = np.trace(G22)
            B2 = np.trace(G22, offset=-1)
            C0 = np.trace(G12)
            Cp = np.trace(G12, offset=-1)  # sum_z x1[z] x2[z-1]
            Cm = np.trace(G12, offset=1)  # sum_z x1[z-1] x2[z]
            ss1 = ((1 - f1) ** 2 + f1**2) * A1 + 2 * f1 * (1 - f1) * B1
            ss2 = ((1 - f2) ** 2 + f2**2) * A2 + 2 * f2 * (1 - f2) * B2
            s12 = (
                ((1 - f1) * (1 - f2) + f1 * f2) * C0
                + (1 - f1) * f2 * Cp
                + f1 * (1 - f2) * Cm
            )
            m11 = ss1 - S1 * S1 / N
            m22 = ss2 - S2 * S2 / N
            m12 = s12 - S1 * S2 / N
            loss += m12**2 + m11 * m22

    return np.array(loss, dtype=np.float32)
